# revision 8
# baseline (speedup 1.0000x reference)
"""ArcFace loss TRN2 kernel: 8-core class-parallel (tensor-parallel over
num_classes), f32r matmul, on-device weight normalization and sum-exp.

kernel(embeddings, labels, weight) -> (loss, output)
  embeddings (512, 512) f32, labels (512,) int, weight (100000, 512) f32
  output (512, 100000) f32 = ArcFace-scaled logits, loss = scalar CE.

Per core c: classes [c*12500, (c+1)*12500), padded to 12544 on device.
Device computes out = (S * emb_n) @ w_n.T for its class shard plus
rowsum(exp(out)); host applies the one-hot phi fix-up (512 cells), merges
the per-core sum-exp, and computes the scalar loss.
"""
import sys

sys.path.insert(0, "/opt/trn_rl_repo")

import math
import numpy as np

B = 512          # batch
E = 512          # embedding dim
C = 100000       # num classes
NCORES = 8
CPC = C // NCORES            # 12500 classes per core
CPC_PAD = 12544              # 98 chunks of 128
CT_SIZES = [512] * 24 + [256]   # class free-tiles per core (24*512 + 256)
PAD_COLS = CPC_PAD - CPC     # 44 zero-padded class columns per core

S = 30.0
MARGIN = 0.5
COS_M = math.cos(MARGIN)
SIN_M = math.sin(MARGIN)
TH = math.cos(math.pi - MARGIN)
MM = math.sin(math.pi - MARGIN) * MARGIN

_CACHE = {}


def _build(out_dma=None):
    import os
    if out_dma is None:
        out_dma = os.environ.get("KERNEL_OUT_DMA", "gpsimd")
    import concourse.bass as bass
    import concourse.tile as tile
    from concourse import bacc, mybir
    from concourse.masks import make_identity

    f32 = mybir.dt.float32
    f32r = mybir.dt.float32r
    P = 128
    Exp = mybir.ActivationFunctionType.Exp
    Ln = mybir.ActivationFunctionType.Ln
    Copy = mybir.ActivationFunctionType.Copy

    # class groups per core: 2x512 (fast pipeline fill) + 11x1024 + 256
    GROUPS = ([(0, 512), (512, 512)]
              + [(1024 + g * 1024, 1024) for g in range(11)]
              + [(12288, 256)])

    nc = bacc.Bacc(None)
    emb = nc.declare_dram_parameter("emb", [B, E], f32, isOutput=False)
    w = nc.declare_dram_parameter("w", [CPC, E], f32, isOutput=False)
    out = nc.declare_dram_parameter("out", [B, CPC_PAD], f32, isOutput=True)
    sumexp = nc.declare_dram_parameter("sumexp", [B], f32, isOutput=True)

    out_v = out.rearrange("(m p) c -> p m c", p=P)        # (128, 4, 12544)
    emb_v = emb.rearrange("(t p) e -> p t e", p=P)        # (128, 4, 512)

    with tile.TileContext(nc) as tc:
        with (
            tc.tile_pool(name="persist", bufs=1) as persist,
            tc.tile_pool(name="wstage", bufs=3) as wstage_pool,
            tc.tile_pool(name="wn", bufs=2) as wn_pool,
            tc.tile_pool(name="wt", bufs=2) as wt_pool,
            tc.tile_pool(name="ostage", bufs=2) as ostage_pool,
            tc.tile_pool(name="scratch", bufs=2) as scratch,
            tc.tile_pool(name="small", bufs=4) as small,
            tc.tile_pool(name="pst", bufs=2, space="PSUM") as pst_pool,
            tc.tile_pool(name="pso", bufs=2, space="PSUM") as pso_pool,
        ):
            out_eng = {"gpsimd": nc.gpsimd, "sync": nc.sync,
                       "scalar": nc.scalar}[out_dma]
            ident_f = persist.tile([P, P], f32)
            make_identity(nc, ident_f)
            ident = persist.tile([P, P], f32r)
            nc.vector.tensor_copy(ident[:], ident_f[:])
            eps24 = persist.tile([P, 1], f32)
            nc.vector.memset(eps24[:], 1e-24)

            # ---- embeddings: load, l2-normalize rows, scale by S, transpose
            emb_sb = persist.tile([P, 4, E], f32)
            nc.sync.dma_start(emb_sb[:], emb_v[:])
            ess = persist.tile([P, 4], f32)
            for t in range(4):
                sq_scr = scratch.tile([P, E], f32, tag="sq")
                nc.vector.scalar_tensor_tensor(
                    out=sq_scr[:], in0=emb_sb[:, t, :], scalar=1.0,
                    in1=emb_sb[:, t, :],
                    op0=mybir.AluOpType.mult, op1=mybir.AluOpType.mult,
                    accum_out=ess[:, t:t + 1],
                )
            elog = persist.tile([P, 4], f32)
            nc.scalar.activation(elog[:], ess[:], Ln, bias=eps24[:])
            ers = persist.tile([P, 4], f32)
            nc.scalar.activation(ers[:], elog[:], Exp, scale=-0.5)
            embS = persist.tile([P, 4, E], f32r)
            for t in range(4):
                nc.vector.tensor_scalar(
                    out=embS[:, t, :], in0=emb_sb[:, t, :],
                    scalar1=ers[:, t:t + 1], scalar2=S,
                    op0=mybir.AluOpType.mult, op1=mybir.AluOpType.mult,
                )
            embT = persist.tile([P, 4, B], f32r)   # [e%128, e//128, b]
            for kh in range(2):
                ps_e = pst_pool.tile([P, 2 * B], f32, tag="tpose")
                for kk in range(2):
                    k = kh * 2 + kk
                    for t in range(4):
                        nc.tensor.matmul(
                            ps_e[:, kk * B + t * P:kk * B + (t + 1) * P].bitcast(f32r),
                            embS[:, t, k * P:(k + 1) * P],
                            ident[:],
                            is_transpose=True,
                        )
                nc.vector.tensor_copy(
                    embT[:, kh * 2:kh * 2 + 2, :].rearrange("p k b -> p (k b)"),
                    ps_e[:].bitcast(f32r))

            # ---- expsum collector: [p, m*16 + g]
            expsums = persist.tile([P, 4 * 16], f32)

            # ---- main loop over 1024-class groups, software-pipelined:
            # stage A(g): DMA in + sumsq + rsqrt chain (DVE/ACT small ops)
            # stage B(g): scale, transpose, wT copy, matmul, exp, DMA out
            state = {}

            def stage_a(g):
                col, GSZ = GROUPS[g]
                NCH = GSZ // P
                w_stage = wstage_pool.tile([P, 8, E], f32, tag="wstage")
                if col + GSZ <= CPC:
                    nc.sync.dma_start(
                        w_stage[:, :GSZ // P, :],
                        w[col:col + GSZ].rearrange("(j p) e -> p j e", p=P),
                    )
                else:
                    nc.vector.memset(w_stage[:, 1, :], 0.0)
                    nc.sync.dma_start(w_stage[:, 0, :], w[col:col + P, :])
                    nc.sync.dma_start(
                        w_stage[:CPC - col - P, 1, :], w[col + P:CPC, :]
                    )
                ssq = small.tile([P, 8], f32, tag="ssq")
                for j in range(NCH):
                    sq_scr = scratch.tile([P, E], f32, tag="sq")
                    nc.vector.scalar_tensor_tensor(
                        out=sq_scr[:], in0=w_stage[:, j, :], scalar=1.0,
                        in1=w_stage[:, j, :],
                        op0=mybir.AluOpType.mult, op1=mybir.AluOpType.mult,
                        accum_out=ssq[:, j:j + 1],
                    )
                wlog = small.tile([P, 8], f32, tag="wlog")
                nc.scalar.activation(wlog[:, :NCH], ssq[:, :NCH], Ln, bias=eps24[:])
                wrs = small.tile([P, 8], f32, tag="wrs")
                nc.scalar.activation(wrs[:, :NCH], wlog[:, :NCH], Exp, scale=-0.5)
                state[g] = (w_stage, wrs)

            def stage_b1(g):
                col, GSZ = GROUPS[g]
                NCH = GSZ // P
                w_stage, wrs = state.pop(g)
                wn = wn_pool.tile([P, 8, E], f32r, tag="wn")
                for j in range(NCH):
                    nc.vector.tensor_scalar(
                        out=wn[:, j, :], in0=w_stage[:, j, :],
                        scalar1=wrs[:, j:j + 1], scalar2=None,
                        op0=mybir.AluOpType.mult,
                    )

                wT = wt_pool.tile([P, 4, 1024], f32r, tag="wt")
                n_half = GSZ // 512 if GSZ >= 512 else 1
                c_half = min(GSZ, 512)
                for half in range(n_half):
                    for kh in range(2):
                        ps_t = pst_pool.tile([P, 1024], f32, tag="tpose")
                        for kk in range(2):
                            k = kh * 2 + kk
                            for j in range(c_half // P):
                                jj = half * 4 + j
                                nc.tensor.matmul(
                                    ps_t[:, kk * 512 + j * P:kk * 512 + (j + 1) * P].bitcast(f32r),
                                    wn[:, jj, k * P:(k + 1) * P],
                                    ident[:],
                                    is_transpose=True,
                                )
                        copy_eng = nc.vector if (half == 0 and kh == 0) else nc.scalar
                        if c_half == 512:
                            dst = wT[:, kh * 2:kh * 2 + 2,
                                     half * 512:(half + 1) * 512]
                            src_ap = ps_t[:].rearrange("p (k c) -> p k c", k=2)
                        else:
                            dst = wT[:, kh * 2:kh * 2 + 2, :c_half]
                            src_ap = ps_t[:].rearrange("p (k c) -> p k c", k=2)[:, :, :c_half]
                        if copy_eng is nc.vector:
                            nc.vector.tensor_copy(dst, src_ap.bitcast(f32r))
                        else:
                            nc.scalar.activation(dst, src_ap, Copy)

                state[("wT", g)] = wT

            def stage_b2(g):
                col, GSZ = GROUPS[g]
                NCH = GSZ // P
                n_half = GSZ // 512 if GSZ >= 512 else 1
                c_half = min(GSZ, 512)
                wT = state.pop(("wT", g))
                o_stage = ostage_pool.tile([P, 4, 1024], f32, tag="ostage")
                for m in range(4):
                    ps_o = pso_pool.tile([P, 1024], f32, tag="mout")
                    for half in range(n_half):
                        for k in range(4):
                            nc.tensor.matmul(
                                ps_o[:, half * 512:half * 512 + c_half],
                                embT[:, k, m * P:(m + 1) * P],
                                wT[:, k, half * 512:half * 512 + c_half],
                                start=(k == 0),
                                stop=(k == 3),
                            )
                    idx = m * 16 + g
                    nc.scalar.activation(
                        o_stage[:, m, :GSZ], ps_o[:, :GSZ], Exp,
                        accum_out=expsums[:, idx:idx + 1],
                    )
                out_eng.dma_start(
                    out_v[:, :, col:col + GSZ], o_stage[:, :, :GSZ]
                )

            NG = len(GROUPS)
            stage_a(0)
            stage_b1(0)
            for g in range(NG):
                if g + 1 < NG:
                    stage_a(g + 1)
                    stage_b1(g + 1)
                stage_b2(g)

            # ---- final row sum-exp reduce + DMA
            se_final = persist.tile([P, 4], f32)
            for m in range(4):
                nc.vector.reduce_sum(
                    se_final[:, m:m + 1],
                    expsums[:, m * 16:m * 16 + len(GROUPS)],
                    axis=mybir.AxisListType.X,
                )
            nc.sync.dma_start(sumexp.rearrange("(m p) -> p m", p=P), se_final[:])

    nc.finalize()
    return nc


def _get_nc():
    import os
    key = "nc_" + os.environ.get("KERNEL_OUT_DMA", "gpsimd")
    if key not in _CACHE:
        _CACHE[key] = _build()
    return _CACHE[key]


def kernel(embeddings, labels, weight):
    from concourse.bass_utils import run_bass_kernel_spmd

    embeddings = np.ascontiguousarray(np.asarray(embeddings, dtype=np.float32))
    weight = np.ascontiguousarray(np.asarray(weight, dtype=np.float32))
    labels_np = np.asarray(labels)

    nc = _get_nc()
    in_maps = [
        {"emb": embeddings, "w": weight[c * CPC:(c + 1) * CPC]}
        for c in range(NCORES)
    ]
    res = run_bass_kernel_spmd(nc, in_maps, core_ids=list(range(NCORES)))

    # device DMAs exp(S*cos); recover the raw logits via log on host
    output = np.concatenate(
        [res.results[c]["out"][:, :CPC] for c in range(NCORES)], axis=1
    )  # (512, 100000) f32, holds exp values
    # per-core row sums of exp incl. the 44 zero-pad columns (exp(0)=1)
    sumexp = np.stack([res.results[c]["sumexp"] for c in range(NCORES)])
    total = sumexp.sum(axis=0, dtype=np.float64) - NCORES * float(PAD_COLS)

    rows = np.arange(B)
    lab = labels_np.astype(np.int64)
    expval = output[rows, lab].astype(np.float64)     # exp(S*cos) at labels
    np.log(output, out=output)                        # raw S*cos everywhere

    # one-hot phi fix-up on the 512 label cells
    cos_s = output[rows, lab].astype(np.float64)      # S * cos
    cos = cos_s / S
    sine = np.sqrt(np.clip(1.0 - cos * cos, 0.0, None))
    phi = cos * COS_M - sine * SIN_M
    phi = np.where(cos > TH, phi, cos - MM)
    target = (phi * S)
    output[rows, lab] = target.astype(np.float32)
    tgt32 = output[rows, lab].astype(np.float64)      # post-rounding value
    total = total + np.exp(tgt32) - expval

    logp = tgt32 - np.log(total)
    loss = np.float32(-logp.mean())
    return loss, output

